# revision 19
# baseline (speedup 1.0000x reference)
"""Trainium2 Bass kernel for the per-feature grouped MLP (SuperLinear/GLU x2).

Math (per feature d of D=2048, batch B=512, M=32, H=64):
  x1 = state[:, d, :] @ w1a[:, :, d] / Ta + b1a[d]      [B, 128]
  h  = x1[:, :64] * sigmoid(x1[:, 64:])                 [B, 64]
  x2 = h @ w1b[:, :, d] / Tb + b1b[d]                   [B, 2]
  out[:, d] = x2[:, 0] * sigmoid(x2[:, 1])

Sharding: D split across 8 cores (embarrassingly parallel), 256 features/core.

Device dataflow per core: one software-pipelined loop over gens (4 features).
Steady-state cadence is bound by the DVE GLU1 multiply (~1.2us/gen at 1x,
PSUM fp32 src). Per gen:
  PE:   G-rounds (2 MMs per feature-pair, 32x64 array tiles), A-round
        (4 MMs), then the 8 MM2 matmuls of gen g-K (K-gen delay so the
        x2 dependency is already satisfied when PE reaches them).
  ACT:  sigmoid per feature-pair [128,512] PSUM->SBUF.
  DVE:  x2 = A * sig [128,1024] fp16 out.
PSUM budget (16KB/partition = 8 banks): A gen-tiles [128,1024]x2 (8KB)
+ G pair-tiles [128,512]x3 (6KB) + MM2 quad [128,512]x1 (2KB).
Separate tags keep buffer reuse A->A / G->G so the sigmoid chain is never
gated by the (later) A-tile free, and the MM2 quad never aliases MM1 banks.
Every window of 16 gens: GLU2 (strided sigmoid + mult over the quad) and
one 3D-AP output DMA.

Feature->partition convention (matches host prep): gen parity gi swaps the
pair rows (f_odd on top for odd gens); the host-built w2 block-diag quads
compensate.
"""

import numpy as np

_CACHE = {}


def _build_nc(B, DL, M, H, window, k_delay, use_ba, use_bg, use_bq):
    import concourse.bass as bass
    import concourse.mybir as mybir
    from concourse import bacc
    from concourse.tile import TileContext

    f32 = mybir.dt.float32
    f16 = mybir.dt.float16
    H2 = 2 * H
    NGEN = DL // 4  # gens of 4 features
    assert NGEN % window == 0 and NGEN % 2 == 0
    NB = B // 128  # b-chunks for MM2
    QR = 8 * window  # quad cols per b-chunk region (2*window pairs x 4)
    FW = 4 * window  # features (output cols) per window

    nc = bacc.Bacc("TRN2", target_bir_lowering=False)

    # st: [128=(j,m), NGEN*B]; w: [128=(j,m), NGEN*128=(gen,(wa|wg))]
    st_d = nc.dram_tensor("st", [128, NGEN * B], f16, kind="ExternalInput")
    w_d = nc.dram_tensor("w", [128, NGEN * H2], f16, kind="ExternalInput")
    # w2 quad weights, window-major: [nwin*128, 32*4]
    w2_d = nc.dram_tensor("w2", [(NGEN // window) * H2, 2 * window * 4], f16,
                          kind="ExternalInput")
    if use_bg:
        bg_d = nc.dram_tensor("bg", [DL, H], f32, kind="ExternalInput")
    if use_ba:
        ba_d = nc.dram_tensor("ba", [DL, H], f32, kind="ExternalInput")
    if use_bq:
        bq_d = nc.dram_tensor("bq", [DL // 2, 4], f32, kind="ExternalInput")
    out_d = nc.dram_tensor("out", [B, DL], f16, kind="ExternalOutput")

    Sig = mybir.ActivationFunctionType.Sigmoid
    Mult = mybir.AluOpType.mult

    with TileContext(nc) as tc:
        with tc.tile_pool(name="sb", bufs=4) as sb, \
             tc.tile_pool(name="ps", bufs=1, space="PSUM") as ps:
            if use_bq:
                bq_t = sb.tile([1, DL * 2], f32, tag="bq", bufs=1, name="bqt")
                ones_t = sb.tile([1, 128], f16, tag="ones", bufs=1,
                                 name="onest")
                nc.sync.dma_start(out=bq_t,
                                  in_=bq_d.rearrange("p q -> 1 (p q)"))
                nc.vector.memset(ones_t, 1.0)

            # warm up the sigmoid table while the first DMAs run
            warm = sb.tile([1, 8], f32, tag="warm", bufs=1, name="warm")
            nc.vector.memset(warm, 0.0)
            nc.scalar.activation(out=warm, in_=warm, func=Sig)

            st_t = {}
            w_t = {}

            def dma_supergen(sg, split=1, gsplit=1):
                # split>1 chunks the transfer by partition groups (and
                # gsplit=2 additionally by gen within the super-gen) so the
                # first MM1 (which only reads rows 32j..32j+32 of one gen)
                # can start before the whole super-gen has landed.
                g0 = 2 * sg
                s = sb.tile([128, 2 * B], f16, tag="st", bufs=6,
                            name=f"st{sg}")
                pc = 128 // split
                for gc in range(gsplit):
                    cs = slice(gc * B, (gc + 1) * B) if gsplit == 2 \
                        else slice(0, 2 * B)
                    dcs = slice((g0 + gc) * B, (g0 + gc + 1) * B) \
                        if gsplit == 2 else slice(g0 * B, (g0 + 2) * B)
                    for c in range(split):
                        rs = slice(c * pc, (c + 1) * pc)
                        nc.sync.dma_start(out=s[rs, cs], in_=st_d[rs, dcs])
                st_t[sg] = s
                w = sb.tile([128, 2 * H2], f16, tag="w", bufs=6,
                            name=f"w{sg}")
                for gc in range(gsplit):
                    cs = slice(gc * H2, (gc + 1) * H2) if gsplit == 2 \
                        else slice(0, 2 * H2)
                    dcs = slice((g0 + gc) * H2, (g0 + gc + 1) * H2) \
                        if gsplit == 2 else slice(g0 * H2, (g0 + 2) * H2)
                    for c in range(split):
                        rs = slice(c * pc, (c + 1) * pc)
                        nc.gpsimd.dma_start(out=w[rs, cs], in_=w_d[rs, dcs])
                w_t[sg] = w

            w2_t = {}

            def dma_w2(wi):
                t = sb.tile([H2, 2 * window * 4], f16, tag="w2", bufs=2,
                            name=f"w2t{wi}")
                nc.gpsimd.dma_start(out=t, in_=w2_d[wi * H2:(wi + 1) * H2, :])
                w2_t[wi] = t

            # Prefetch. The first super-gen is chunked so gen 0's first
            # matmuls (rows 0..32, cols 0..B) unblock as early as possible:
            # w chunks first (small, on the gpsimd queue), then st row-groups
            # of gen 0, then gen 1's columns.
            s0 = sb.tile([128, 2 * B], f16, tag="st", bufs=6, name="st0")
            w0 = sb.tile([128, 2 * H2], f16, tag="w", bufs=6, name="w0")
            for c in range(4):
                rs = slice(32 * c, 32 * c + 32)
                nc.gpsimd.dma_start(out=w0[rs, :], in_=w_d[rs, 0:2 * H2])
            for c in range(4):
                rs = slice(32 * c, 32 * c + 32)
                nc.sync.dma_start(out=s0[rs, 0:B], in_=st_d[rs, 0:B])
            nc.sync.dma_start(out=s0[:, B:2 * B], in_=st_d[:, B:2 * B])
            st_t[0] = s0
            w_t[0] = w0
            dma_supergen(1, split=2)
            dma_w2(0)

            x2_list = {}
            quad = {}

            def mm2_gen(g):
                """8 MM2 matmuls for gen g into its window's quad tile."""
                wi = g // window
                if g % window == 0:
                    quad[wi] = ps.tile([128, NB * QR], f32, tag="q", bufs=1,
                                       name=f"quad{wi}")
                    if wi + 1 < NGEN // window:
                        dma_w2(wi + 1)
                q = quad[wi]
                x2g = x2_list.pop(g)
                wt = w2_t[wi]
                for p in range(2):
                    pl = (g % window) * 2 + p  # pair idx in window
                    for bc in range(NB):
                        qo = bc * QR + pl * 4
                        nc.tensor.matmul(
                            out=q[:, qo:qo + 4],
                            lhsT=x2g[:, p * B + bc * 128:
                                     p * B + (bc + 1) * 128],
                            rhs=wt[:, pl * 4:pl * 4 + 4],
                            start=True, stop=not use_bq)
                        if use_bq:
                            pg = (g * 2 + p)  # global pair
                            nc.tensor.matmul(
                                out=q[:, qo:qo + 4],
                                lhsT=ones_t,
                                rhs=bq_t[:, pg * 4:pg * 4 + 4],
                                start=False, stop=True)

            def glu2_window(wi):
                # ACT stages both strided quad halves to SBUF f16 (freeing
                # the single quad bank early for the next window); the DVE
                # product then runs all-f16 in 2x mode.
                q = quad[wi]
                sig2 = sb.tile([128, NB * QR // 2], f16, tag="sig2",
                               bufs=2, name=f"sig2{wi}")
                o1 = sb.tile([128, NB * QR // 2], f16, tag="o1", bufs=2,
                             name=f"o1{wi}")
                o_t = sb.tile([128, NB * QR // 2], f16, tag="o", bufs=2,
                              name=f"o{wi}")
                nc.scalar.activation(
                    out=sig2, in_=q[:, 1:NB * QR:2], func=Sig)
                nc.scalar.copy(out=o1, in_=q[:, 0:NB * QR:2])
                nc.vector.tensor_tensor(out=o_t, in0=o1, in1=sig2, op=Mult)
                dst = out_d.rearrange("(bc p) d -> p bc d", bc=NB)
                nc.sync.dma_start(
                    out=dst[:, :, wi * FW:(wi + 1) * FW],
                    in_=o_t.rearrange("p (bc d) -> p bc d", bc=NB))

            for g in range(NGEN):
                sg, gi = g // 2, g % 2
                if gi == 0 and sg + 2 <= NGEN // 2 - 1:
                    dma_supergen(sg + 2)
                st = st_t[sg]
                wt = w_t[sg]

                # MM1. Per feature j: K-rows 32j..32j+32, array col-half
                # cp = 64*((j%2)^gi) (parity XOR gen -> consecutive gens use
                # complementary tiles). G first (feeds sigmoid; freed early
                # by ACT), then A (freed late by the DVE mult). The G matmuls
                # are chunked along B: completion latency scales with the
                # output free-dim, and G completion sits on the sigmoid
                # critical chain (sigma -> G-bank free -> G matmul -> sigma).
                Gp = []
                for p in range(2):
                    gt = ps.tile([128, B], f32, tag="G", bufs=3,
                                 name=f"G{g}p{p}")
                    for j in (2 * p, 2 * p + 1):
                        rs = slice(32 * j, 32 * j + 32)
                        cp = 64 * ((j % 2) ^ gi)
                        for c in range(2):
                            cb = c * (B // 2)
                            nc.tensor.matmul(
                                out=gt[cp:cp + 64, cb:cb + B // 2],
                                lhsT=wt[rs, gi * H2 + H:gi * H2 + H2],
                                rhs=st[rs, gi * B + cb:gi * B + cb + B // 2],
                                start=True, stop=True,
                                tile_position=(32 * j, cp))
                    Gp.append(gt)

                # interleaved MM2 for gen g-k_delay (x2 ready long ago);
                # emitted between the G and A rounds so it never delays the
                # next gen's G issue.
                gd = g - k_delay
                if gd >= 0:
                    mm2_gen(gd)
                    if gd % window == window - 1:
                        glu2_window(gd // window)

                A = ps.tile([128, 2 * B], f32, tag="A", bufs=2, name=f"A{g}")
                for j in range(4):
                    rs = slice(32 * j, 32 * j + 32)
                    cp = 64 * ((j % 2) ^ gi)
                    fb = B * (j // 2)
                    nc.tensor.matmul(
                        out=A[cp:cp + 64, fb:fb + B],
                        lhsT=wt[rs, gi * H2:gi * H2 + H],
                        rhs=st[rs, gi * B:(gi + 1) * B],
                        start=True, stop=True,
                        tile_position=(32 * j, cp))

                # GLU1: sigmoid per pair (ACT), multiply per gen (DVE)
                sig = sb.tile([128, 2 * B], f16, tag="sig", bufs=6,
                              name=f"sig{g}")
                for p in range(2):
                    if use_bg:
                        bg_t = sb.tile([128, 2], f32, tag="bg",
                                       name=f"bg{g}p{p}")
                        nc.sync.dma_start(
                            out=bg_t,
                            in_=bg_d[4 * g + 2 * p:4 * g + 2 * p + 2, :]
                            .rearrange("(p two) h -> (two h) p", two=2))
                        nc.scalar.activation(
                            out=sig[:, p * B:(p + 1) * B], in_=Gp[p],
                            func=Sig, bias=bg_t[:, 0:1])
                    else:
                        nc.scalar.activation(
                            out=sig[:, p * B:(p + 1) * B], in_=Gp[p],
                            func=Sig)
                # full buffering: no reuse -> the mult never waits on MM2
                x2 = sb.tile([128, 2 * B], f16, tag="x2", bufs=NGEN,
                             name=f"x2{g}")
                if use_ba:
                    ba_t = sb.tile([128, 2], f32, tag="ba", name=f"ba{g}")
                    nc.sync.dma_start(
                        out=ba_t,
                        in_=ba_d[4 * g:4 * g + 4, :].rearrange(
                            "(p two) h -> (two h) p", two=2))
                    for k in range(2):
                        nc.vector.scalar_tensor_tensor(
                            out=x2[:, k * B:(k + 1) * B],
                            in0=A[:, k * B:(k + 1) * B],
                            scalar=ba_t[:, k:k + 1],
                            in1=sig[:, k * B:(k + 1) * B],
                            op0=mybir.AluOpType.add, op1=Mult)
                else:
                    nc.vector.tensor_tensor(out=x2, in0=A, in1=sig, op=Mult)
                x2_list[g] = x2

            # drain the last k_delay gens
            for gd in range(NGEN - k_delay, NGEN):
                mm2_gen(gd)
                if gd % window == window - 1:
                    glu2_window(gd // window)
    nc.finalize()
    return nc


def _gen_major(a, NC, NGEN):
    """[D, 32, X] -> per-core [128=(j,m), NGEN*X] with gen-major free dim."""
    D = a.shape[0]
    X = a.shape[2]
    r = a.reshape(NC, NGEN, 4 * 32, X).transpose(0, 2, 1, 3)
    return np.ascontiguousarray(r.reshape(NC, 128, NGEN * X))


def _host_prep(state_trace, w1a, b1a, Ta, w1b, b1b, Tb, NC):
    import ml_dtypes  # noqa: F401  (fp16 is native numpy)

    B, D, M = state_trace.shape
    H2 = w1a.shape[1]
    H = H2 // 2
    DL = D // NC
    window = 16
    NGEN = DL // 4

    Ta_v = float(np.asarray(Ta).reshape(-1)[0])
    Tb_v = float(np.asarray(Tb).reshape(-1)[0])

    # state: [B, D, M] -> [D, M, B] fp16 -> gen-major
    st = np.asarray(state_trace, np.float32).transpose(1, 2, 0)
    st = _gen_major(st.astype(np.float16), NC, NGEN)

    # w1a: [M, 2H, D]/Ta -> [D, M, 2H] fp16 (cols: wa | wg) -> gen-major
    w1aT = (np.asarray(w1a, np.float32).transpose(2, 0, 1)
            * np.float32(1.0 / Ta_v))
    w = _gen_major(w1aT.astype(np.float16), NC, NGEN)

    # w2 block-diag quads: [D/2 pairs, 2H, 4], cols (c0f0,c1f0,c0f1,c1f1)
    # where f0 = even feature of the pair. For pairs of ODD gens the x2
    # partition blocks are swapped (f_odd on top), so swap the row blocks.
    w1bT = (np.asarray(w1b, np.float32).transpose(2, 0, 1)
            * np.float32(1.0 / Tb_v))  # [D, H, 2]
    w2q = np.zeros((D // 2, H2, 4), np.float32)
    pr = np.arange(D // 2)
    odd = (pr // 2) % 2 == 1  # pair's gen parity
    ev = ~odd
    w2q[ev, :H, 0] = w1bT[0::2][ev, :, 0]
    w2q[ev, :H, 1] = w1bT[0::2][ev, :, 1]
    w2q[ev, H:, 2] = w1bT[1::2][ev, :, 0]
    w2q[ev, H:, 3] = w1bT[1::2][ev, :, 1]
    w2q[odd, H:, 0] = w1bT[0::2][odd, :, 0]
    w2q[odd, H:, 1] = w1bT[0::2][odd, :, 1]
    w2q[odd, :H, 2] = w1bT[1::2][odd, :, 0]
    w2q[odd, :H, 3] = w1bT[1::2][odd, :, 1]
    nwin = NGEN // window
    w2q = w2q.reshape(NC, nwin, 2 * window, H2, 4).transpose(0, 1, 3, 2, 4)
    w2q = np.ascontiguousarray(
        w2q.reshape(NC, nwin * H2, 2 * window * 4)).astype(np.float16)

    # biases (device order: for odd gens the pair rows are swapped)
    b1a_f = np.asarray(b1a, np.float32).reshape(D, H2) * np.float32(1 / Ta_v)
    gperm = np.arange(D).reshape(-1, 4)
    gperm[1::2] = gperm[1::2][:, [1, 0, 3, 2]]
    gperm = gperm.reshape(-1)
    ba = np.ascontiguousarray(b1a_f[gperm, :H])
    bg = np.ascontiguousarray(b1a_f[gperm, H:])
    b1b_f = np.asarray(b1b, np.float32).reshape(D, 2) * np.float32(1 / Tb_v)
    bq = np.zeros((D // 2, 4), np.float32)
    bq[:, 0] = b1b_f[0::2, 0]
    bq[:, 1] = b1b_f[0::2, 1]
    bq[:, 2] = b1b_f[1::2, 0]
    bq[:, 3] = b1b_f[1::2, 1]

    use_ba = bool(np.any(ba))
    use_bg = bool(np.any(bg))
    use_bq = bool(np.any(bq))

    in_maps = []
    for c in range(NC):
        ds = slice(c * DL, (c + 1) * DL)
        m = {"st": st[c], "w": w[c], "w2": w2q[c]}
        if use_bg:
            m["bg"] = np.ascontiguousarray(bg[ds])
        if use_ba:
            m["ba"] = np.ascontiguousarray(ba[ds])
        if use_bq:
            m["bq"] = np.ascontiguousarray(bq[c * DL // 2:(c + 1) * DL // 2])
        in_maps.append(m)
    cfg = dict(B=B, DL=DL, M=M, H=H, window=window, k_delay=3,
               use_ba=use_ba, use_bg=use_bg, use_bq=use_bq)
    return in_maps, cfg


def kernel(state_trace, w1a, b1a, Ta, w1b, b1b, Tb):
    from concourse.bass_utils import run_bass_kernel_spmd

    NC = 8
    B, D, M = state_trace.shape
    in_maps, cfg = _host_prep(state_trace, w1a, b1a, Ta, w1b, b1b, Tb, NC)

    key = tuple(sorted(cfg.items()))
    if key not in _CACHE:
        _CACHE[key] = _build_nc(**cfg)
    nc = _CACHE[key]

    res = run_bass_kernel_spmd(nc, in_maps, core_ids=list(range(NC)))
    out = np.empty((B, D), np.float32)
    DL = D // NC
    for c in range(NC):
        out[:, c * DL:(c + 1) * DL] = np.asarray(
            res.results[c]["out"], np.float32)
    return out


# revision 21
# speedup vs baseline: 1.0159x; 1.0159x over previous
"""Trainium2 Bass kernel for the per-feature grouped MLP (SuperLinear/GLU x2).

Math (per feature d of D=2048, batch B=512, M=32, H=64):
  x1 = state[:, d, :] @ w1a[:, :, d] / Ta + b1a[d]      [B, 128]
  h  = x1[:, :64] * sigmoid(x1[:, 64:])                 [B, 64]
  x2 = h @ w1b[:, :, d] / Tb + b1b[d]                   [B, 2]
  out[:, d] = x2[:, 0] * sigmoid(x2[:, 1])

Sharding: D split across 8 cores (embarrassingly parallel), 256 features/core.

Device dataflow per core: one software-pipelined loop over gens (4 features).
Steady-state cadence is bound by the DVE GLU1 multiply (~1.2us/gen at 1x,
PSUM fp32 src). Per gen:
  PE:   G-rounds (2 MMs per feature-pair, 32x64 array tiles), A-round
        (4 MMs), then the 8 MM2 matmuls of gen g-K (K-gen delay so the
        x2 dependency is already satisfied when PE reaches them).
  ACT:  sigmoid per feature-pair [128,512] PSUM->SBUF.
  DVE:  x2 = A * sig [128,1024] fp16 out.
PSUM budget (16KB/partition = 8 banks): A gen-tiles [128,1024]x2 (8KB)
+ G pair-tiles [128,512]x3 (6KB) + MM2 quad [128,512]x1 (2KB).
Separate tags keep buffer reuse A->A / G->G so the sigmoid chain is never
gated by the (later) A-tile free, and the MM2 quad never aliases MM1 banks.
Every window of 16 gens: GLU2 (strided sigmoid + mult over the quad) and
one 3D-AP output DMA.

Feature->partition convention (matches host prep): gen parity gi swaps the
pair rows (f_odd on top for odd gens); the host-built w2 block-diag quads
compensate.
"""

import numpy as np

_CACHE = {}


def _build_nc(B, DL, M, H, window, k_delay, use_ba, use_bg, use_bq):
    import concourse.bass as bass
    import concourse.mybir as mybir
    from concourse import bacc
    from concourse.tile import TileContext

    f32 = mybir.dt.float32
    f16 = mybir.dt.float16
    H2 = 2 * H
    NGEN = DL // 4  # gens of 4 features
    assert NGEN % window == 0 and NGEN % 2 == 0
    NB = B // 128  # b-chunks for MM2
    QR = 8 * window  # quad cols per b-chunk region (2*window pairs x 4)
    FW = 4 * window  # features (output cols) per window

    nc = bacc.Bacc("TRN2", target_bir_lowering=False)

    # st: [128=(j,m), NGEN*B]; w: [128=(j,m), NGEN*128=(gen,(wa|wg))]
    st_d = nc.dram_tensor("st", [128, NGEN * B], f16, kind="ExternalInput")
    w_d = nc.dram_tensor("w", [128, NGEN * H2], f16, kind="ExternalInput")
    # w2 quad weights, window-major: [nwin*128, 32*4]
    w2_d = nc.dram_tensor("w2", [(NGEN // window) * H2, 2 * window * 4], f16,
                          kind="ExternalInput")
    if use_bg:
        bg_d = nc.dram_tensor("bg", [DL, H], f32, kind="ExternalInput")
    if use_ba:
        ba_d = nc.dram_tensor("ba", [DL, H], f32, kind="ExternalInput")
    if use_bq:
        bq_d = nc.dram_tensor("bq", [DL // 2, 4], f32, kind="ExternalInput")
    out_d = nc.dram_tensor("out", [B, DL], f32, kind="ExternalOutput")

    Sig = mybir.ActivationFunctionType.Sigmoid
    Mult = mybir.AluOpType.mult

    with TileContext(nc) as tc:
        with tc.tile_pool(name="sb", bufs=4) as sb, \
             tc.tile_pool(name="ps", bufs=1, space="PSUM") as ps:
            if use_bq:
                bq_t = sb.tile([1, DL * 2], f32, tag="bq", bufs=1, name="bqt")
                ones_t = sb.tile([1, 128], f16, tag="ones", bufs=1,
                                 name="onest")
                nc.sync.dma_start(out=bq_t,
                                  in_=bq_d.rearrange("p q -> 1 (p q)"))
                nc.vector.memset(ones_t, 1.0)

            # warm up the sigmoid table while the first DMAs run
            warm = sb.tile([1, 8], f32, tag="warm", bufs=1, name="warm")
            nc.vector.memset(warm, 0.0)
            nc.scalar.activation(out=warm, in_=warm, func=Sig)

            st_t = {}
            w_t = {}

            def dma_supergen(sg, split=1, gsplit=1):
                # split>1 chunks the transfer by partition groups (and
                # gsplit=2 additionally by gen within the super-gen) so the
                # first MM1 (which only reads rows 32j..32j+32 of one gen)
                # can start before the whole super-gen has landed.
                g0 = 2 * sg
                s = sb.tile([128, 2 * B], f16, tag="st", bufs=6,
                            name=f"st{sg}")
                pc = 128 // split
                for gc in range(gsplit):
                    cs = slice(gc * B, (gc + 1) * B) if gsplit == 2 \
                        else slice(0, 2 * B)
                    dcs = slice((g0 + gc) * B, (g0 + gc + 1) * B) \
                        if gsplit == 2 else slice(g0 * B, (g0 + 2) * B)
                    for c in range(split):
                        rs = slice(c * pc, (c + 1) * pc)
                        nc.sync.dma_start(out=s[rs, cs], in_=st_d[rs, dcs])
                st_t[sg] = s
                w = sb.tile([128, 2 * H2], f16, tag="w", bufs=6,
                            name=f"w{sg}")
                for gc in range(gsplit):
                    cs = slice(gc * H2, (gc + 1) * H2) if gsplit == 2 \
                        else slice(0, 2 * H2)
                    dcs = slice((g0 + gc) * H2, (g0 + gc + 1) * H2) \
                        if gsplit == 2 else slice(g0 * H2, (g0 + 2) * H2)
                    for c in range(split):
                        rs = slice(c * pc, (c + 1) * pc)
                        nc.gpsimd.dma_start(out=w[rs, cs], in_=w_d[rs, dcs])
                w_t[sg] = w

            w2_t = {}

            def dma_w2(wi):
                t = sb.tile([H2, 2 * window * 4], f16, tag="w2", bufs=2,
                            name=f"w2t{wi}")
                nc.gpsimd.dma_start(out=t, in_=w2_d[wi * H2:(wi + 1) * H2, :])
                w2_t[wi] = t

            # Prefetch. The first super-gen is chunked so gen 0's first
            # matmuls (rows 0..32, cols 0..B) unblock as early as possible:
            # w chunks first (small, on the gpsimd queue), then st row-groups
            # of gen 0, then gen 1's columns.
            s0 = sb.tile([128, 2 * B], f16, tag="st", bufs=6, name="st0")
            w0 = sb.tile([128, 2 * H2], f16, tag="w", bufs=6, name="w0")
            for c in range(4):
                rs = slice(32 * c, 32 * c + 32)
                nc.gpsimd.dma_start(out=w0[rs, :], in_=w_d[rs, 0:2 * H2])
            for c in range(4):
                rs = slice(32 * c, 32 * c + 32)
                nc.sync.dma_start(out=s0[rs, 0:B], in_=st_d[rs, 0:B])
            nc.sync.dma_start(out=s0[:, B:2 * B], in_=st_d[:, B:2 * B])
            st_t[0] = s0
            w_t[0] = w0
            dma_supergen(1, split=2)
            dma_w2(0)

            x2_list = {}
            quad = {}

            def mm2_gen(g):
                """8 MM2 matmuls for gen g into its window's quad tile."""
                wi = g // window
                if g % window == 0:
                    quad[wi] = ps.tile([128, NB * QR], f32, tag="q", bufs=1,
                                       name=f"quad{wi}")
                    if wi + 1 < NGEN // window:
                        dma_w2(wi + 1)
                q = quad[wi]
                x2g = x2_list.pop(g)
                wt = w2_t[wi]
                for p in range(2):
                    pl = (g % window) * 2 + p  # pair idx in window
                    for bc in range(NB):
                        qo = bc * QR + pl * 4
                        nc.tensor.matmul(
                            out=q[:, qo:qo + 4],
                            lhsT=x2g[:, p * B + bc * 128:
                                     p * B + (bc + 1) * 128],
                            rhs=wt[:, pl * 4:pl * 4 + 4],
                            start=True, stop=not use_bq)
                        if use_bq:
                            pg = (g * 2 + p)  # global pair
                            nc.tensor.matmul(
                                out=q[:, qo:qo + 4],
                                lhsT=ones_t,
                                rhs=bq_t[:, pg * 4:pg * 4 + 4],
                                start=False, stop=True)

            def glu2_window(wi):
                q = quad[wi]
                sig2 = sb.tile([128, NB * QR // 2], f32, tag="sig2",
                               bufs=2, name=f"sig2{wi}")
                o_t = sb.tile([128, NB * QR // 2], f32, tag="o", bufs=2,
                              name=f"o{wi}")
                nc.scalar.activation(
                    out=sig2, in_=q[:, 1:NB * QR:2], func=Sig)
                nc.vector.tensor_tensor(
                    out=o_t, in0=q[:, 0:NB * QR:2], in1=sig2, op=Mult)
                dst = out_d.rearrange("(bc p) d -> p bc d", bc=NB)
                nc.sync.dma_start(
                    out=dst[:, :, wi * FW:(wi + 1) * FW],
                    in_=o_t.rearrange("p (bc d) -> p bc d", bc=NB))

            for g in range(NGEN):
                sg, gi = g // 2, g % 2
                if gi == 0 and sg + 2 <= NGEN // 2 - 1:
                    dma_supergen(sg + 2)
                st = st_t[sg]
                wt = w_t[sg]

                # MM1. Per feature j: K-rows 32j..32j+32, array col-half
                # cp = 64*((j%2)^gi) (parity XOR gen -> consecutive gens use
                # complementary tiles). G first (feeds sigmoid; freed early
                # by ACT), then A (freed late by the DVE mult). The G matmuls
                # are chunked along B: completion latency scales with the
                # output free-dim, and G completion sits on the sigmoid
                # critical chain (sigma -> G-bank free -> G matmul -> sigma).
                Gp = []
                for p in range(2):
                    gt = ps.tile([128, B], f32, tag="G", bufs=3,
                                 name=f"G{g}p{p}")
                    for j in (2 * p, 2 * p + 1):
                        rs = slice(32 * j, 32 * j + 32)
                        cp = 64 * ((j % 2) ^ gi)
                        for c in range(2):
                            cb = c * (B // 2)
                            nc.tensor.matmul(
                                out=gt[cp:cp + 64, cb:cb + B // 2],
                                lhsT=wt[rs, gi * H2 + H:gi * H2 + H2],
                                rhs=st[rs, gi * B + cb:gi * B + cb + B // 2],
                                start=True, stop=True,
                                tile_position=(32 * j, cp))
                    Gp.append(gt)

                # interleaved MM2 for gen g-k_delay (x2 ready long ago);
                # emitted between the G and A rounds so it never delays the
                # next gen's G issue.
                gd = g - k_delay
                if gd >= 0:
                    mm2_gen(gd)
                    if gd % window == window - 1:
                        glu2_window(gd // window)

                A = ps.tile([128, 2 * B], f32, tag="A", bufs=2, name=f"A{g}")
                for j in range(4):
                    rs = slice(32 * j, 32 * j + 32)
                    cp = 64 * ((j % 2) ^ gi)
                    fb = B * (j // 2)
                    nc.tensor.matmul(
                        out=A[cp:cp + 64, fb:fb + B],
                        lhsT=wt[rs, gi * H2:gi * H2 + H],
                        rhs=st[rs, gi * B:(gi + 1) * B],
                        start=True, stop=True,
                        tile_position=(32 * j, cp))

                # GLU1: sigmoid per pair (ACT), multiply per gen (DVE)
                sig = sb.tile([128, 2 * B], f16, tag="sig", bufs=6,
                              name=f"sig{g}")
                for p in range(2):
                    if use_bg:
                        bg_t = sb.tile([128, 2], f32, tag="bg",
                                       name=f"bg{g}p{p}")
                        nc.sync.dma_start(
                            out=bg_t,
                            in_=bg_d[4 * g + 2 * p:4 * g + 2 * p + 2, :]
                            .rearrange("(p two) h -> (two h) p", two=2))
                        nc.scalar.activation(
                            out=sig[:, p * B:(p + 1) * B], in_=Gp[p],
                            func=Sig, bias=bg_t[:, 0:1])
                    else:
                        nc.scalar.activation(
                            out=sig[:, p * B:(p + 1) * B], in_=Gp[p],
                            func=Sig)
                # full buffering: no reuse -> the mult never waits on MM2
                x2 = sb.tile([128, 2 * B], f16, tag="x2", bufs=NGEN,
                             name=f"x2{g}")
                if use_ba:
                    ba_t = sb.tile([128, 2], f32, tag="ba", name=f"ba{g}")
                    nc.sync.dma_start(
                        out=ba_t,
                        in_=ba_d[4 * g:4 * g + 4, :].rearrange(
                            "(p two) h -> (two h) p", two=2))
                    for k in range(2):
                        nc.vector.scalar_tensor_tensor(
                            out=x2[:, k * B:(k + 1) * B],
                            in0=A[:, k * B:(k + 1) * B],
                            scalar=ba_t[:, k:k + 1],
                            in1=sig[:, k * B:(k + 1) * B],
                            op0=mybir.AluOpType.add, op1=Mult)
                else:
                    nc.vector.tensor_tensor(out=x2, in0=A, in1=sig, op=Mult)
                x2_list[g] = x2

            # drain the last k_delay gens
            for gd in range(NGEN - k_delay, NGEN):
                mm2_gen(gd)
                if gd % window == window - 1:
                    glu2_window(gd // window)
    nc.finalize()
    return nc


def _gen_major(a, NC, NGEN):
    """[D, 32, X] -> per-core [128=(j,m), NGEN*X] with gen-major free dim."""
    D = a.shape[0]
    X = a.shape[2]
    r = a.reshape(NC, NGEN, 4 * 32, X).transpose(0, 2, 1, 3)
    return np.ascontiguousarray(r.reshape(NC, 128, NGEN * X))


def _host_prep(state_trace, w1a, b1a, Ta, w1b, b1b, Tb, NC):
    import ml_dtypes  # noqa: F401  (fp16 is native numpy)

    B, D, M = state_trace.shape
    H2 = w1a.shape[1]
    H = H2 // 2
    DL = D // NC
    window = 16
    NGEN = DL // 4

    Ta_v = float(np.asarray(Ta).reshape(-1)[0])
    Tb_v = float(np.asarray(Tb).reshape(-1)[0])

    # state: [B, D, M] -> [D, M, B] fp16 -> gen-major
    st = np.asarray(state_trace, np.float32).transpose(1, 2, 0)
    st = _gen_major(st.astype(np.float16), NC, NGEN)

    # w1a: [M, 2H, D]/Ta -> [D, M, 2H] fp16 (cols: wa | wg) -> gen-major
    w1aT = (np.asarray(w1a, np.float32).transpose(2, 0, 1)
            * np.float32(1.0 / Ta_v))
    w = _gen_major(w1aT.astype(np.float16), NC, NGEN)

    # w2 block-diag quads: [D/2 pairs, 2H, 4], cols (c0f0,c1f0,c0f1,c1f1)
    # where f0 = even feature of the pair. For pairs of ODD gens the x2
    # partition blocks are swapped (f_odd on top), so swap the row blocks.
    w1bT = (np.asarray(w1b, np.float32).transpose(2, 0, 1)
            * np.float32(1.0 / Tb_v))  # [D, H, 2]
    w2q = np.zeros((D // 2, H2, 4), np.float32)
    pr = np.arange(D // 2)
    odd = (pr // 2) % 2 == 1  # pair's gen parity
    ev = ~odd
    w2q[ev, :H, 0] = w1bT[0::2][ev, :, 0]
    w2q[ev, :H, 1] = w1bT[0::2][ev, :, 1]
    w2q[ev, H:, 2] = w1bT[1::2][ev, :, 0]
    w2q[ev, H:, 3] = w1bT[1::2][ev, :, 1]
    w2q[odd, H:, 0] = w1bT[0::2][odd, :, 0]
    w2q[odd, H:, 1] = w1bT[0::2][odd, :, 1]
    w2q[odd, :H, 2] = w1bT[1::2][odd, :, 0]
    w2q[odd, :H, 3] = w1bT[1::2][odd, :, 1]
    nwin = NGEN // window
    w2q = w2q.reshape(NC, nwin, 2 * window, H2, 4).transpose(0, 1, 3, 2, 4)
    w2q = np.ascontiguousarray(
        w2q.reshape(NC, nwin * H2, 2 * window * 4)).astype(np.float16)

    # biases (device order: for odd gens the pair rows are swapped)
    b1a_f = np.asarray(b1a, np.float32).reshape(D, H2) * np.float32(1 / Ta_v)
    gperm = np.arange(D).reshape(-1, 4)
    gperm[1::2] = gperm[1::2][:, [1, 0, 3, 2]]
    gperm = gperm.reshape(-1)
    ba = np.ascontiguousarray(b1a_f[gperm, :H])
    bg = np.ascontiguousarray(b1a_f[gperm, H:])
    b1b_f = np.asarray(b1b, np.float32).reshape(D, 2) * np.float32(1 / Tb_v)
    bq = np.zeros((D // 2, 4), np.float32)
    bq[:, 0] = b1b_f[0::2, 0]
    bq[:, 1] = b1b_f[0::2, 1]
    bq[:, 2] = b1b_f[1::2, 0]
    bq[:, 3] = b1b_f[1::2, 1]

    use_ba = bool(np.any(ba))
    use_bg = bool(np.any(bg))
    use_bq = bool(np.any(bq))

    in_maps = []
    for c in range(NC):
        ds = slice(c * DL, (c + 1) * DL)
        m = {"st": st[c], "w": w[c], "w2": w2q[c]}
        if use_bg:
            m["bg"] = np.ascontiguousarray(bg[ds])
        if use_ba:
            m["ba"] = np.ascontiguousarray(ba[ds])
        if use_bq:
            m["bq"] = np.ascontiguousarray(bq[c * DL // 2:(c + 1) * DL // 2])
        in_maps.append(m)
    cfg = dict(B=B, DL=DL, M=M, H=H, window=window, k_delay=3,
               use_ba=use_ba, use_bg=use_bg, use_bq=use_bq)
    return in_maps, cfg


def kernel(state_trace, w1a, b1a, Ta, w1b, b1b, Tb):
    from concourse.bass_utils import run_bass_kernel_spmd

    NC = 8
    B, D, M = state_trace.shape
    in_maps, cfg = _host_prep(state_trace, w1a, b1a, Ta, w1b, b1b, Tb, NC)

    key = tuple(sorted(cfg.items()))
    if key not in _CACHE:
        _CACHE[key] = _build_nc(**cfg)
    nc = _CACHE[key]

    res = run_bass_kernel_spmd(nc, in_maps, core_ids=list(range(NC)))
    out = np.empty((B, D), np.float32)
    DL = D // NC
    for c in range(NC):
        out[:, c * DL:(c + 1) * DL] = np.asarray(
            res.results[c]["out"], np.float32)
    return out
